# revision 25
# baseline (speedup 1.0000x reference)
"""AMSoftmax (norm-free branch) Trainium2 kernel, 8 NeuronCores.

Reference computes, for input x [B,D], label [B], weight [C,D], scalars s,m:
    norm   = ||x||_2 per row                       [B,1]
    cosine = (x/max(norm,eps)) @ (w/max(||w||,eps)).T   [B,C]
    logits = norm * (cosine - m*onehot(label))     [B,C]
    returns (logits, cosine)

Key identity: norm * cosine == x @ w_hat.T exactly, so per output element:
    raw    = x @ w_hat.T          (PSUM, f32)
    cosine = raw * (1/norm)       (per-row scale, ACT)
    logits = raw - norm*m*onehot  (DVE sub against a sparse mask)

Sharding: 2-way over batch x 4-way over classes (8 cores, no collectives;
outputs are disjoint tiles concatenated on host). Per core: x [2048,512],
w_hat [2000,512], outputs [2048,2000] each, stored as bf16.

v6:
- x and w shipped bf16 (host dtype prep): input DMA is 4.2MB, no casts.
- W pairs: sumsq (alternating DVE-batched / ACT+accum), scale-cast on
  DVE, fast is_transpose into bf16 PSUM, one merged copy per pair.
- Quarter-width single-bank PSUM groups, 6-deep psM pool.
- Identity built before the GPSIMD library load; a dummy scatter right
  after the load eats the ~6us hidden IRAM fetch inside the DMA shadow.
- W pair prep for pairs 2-7 rides iterations 0-5 (CATCH=6 so h=1 starts
  after all W is ready); stores on the sync ring strictly after inputs.
- PE warm-up bursts keep the HAM clock gate open through the prologue.
"""

import os
import sys

sys.path.insert(0, "/opt/trn_rl_repo")

import numpy as np

B, D, C = 4096, 512, 8000
NB, NCL = 2, 4  # batch x class core grid
BL, CL = B // NB, C // NCL  # 2048, 2000 per core
RT = BL // 128  # 16 row tiles
KC = D // 128  # 4 contraction chunks
CW = 500  # matmul free-dim chunk (PSUM bank holds 512 f32)
HW_ = 2 * CW  # 1000 columns per half
NH = CL // HW_  # 2 column halves per row tile

OUT_BF16 = os.environ.get("AMS_OUT", "bf16") == "bf16"
WARMUP_MM = int(os.environ.get("AMS_WARMUP", "20"))
WARMUP2_MM = int(os.environ.get("AMS_WARMUP2", "40"))
CATCH = int(os.environ.get("AMS_CATCH", "6"))  # h=1 catch-up offset

_CACHE = {}


def _build():
    import concourse.mybir as mybir
    import concourse.tile as tile
    from concourse import bacc, library_config
    from concourse.masks import make_identity

    f32 = mybir.dt.float32
    i16 = mybir.dt.int16
    bf16 = mybir.dt.bfloat16
    odt = bf16 if OUT_BF16 else f32

    nc = bacc.Bacc()
    x_ext = nc.declare_dram_parameter("x", [BL, D], bf16, isOutput=False)
    w_ext = nc.declare_dram_parameter("w", [CL, D], bf16, isOutput=False)
    labx_ext = nc.declare_dram_parameter("labx", [128, 2 * RT], i16, isOutput=False)
    m_ext = nc.declare_dram_parameter("mvec", [128, 1], f32, isOutput=False)
    logits_ext = nc.declare_dram_parameter("logits", [BL, CL], odt, isOutput=True)
    cosine_ext = nc.declare_dram_parameter("cosine", [BL, CL], odt, isOutput=True)

    WT = (CL + 127) // 128  # 16 w row tiles (last one 80 partitions)

    with tile.TileContext(nc) as tc:
        with (
            tc.tile_pool(name="persist", bufs=1) as persist,
            tc.tile_pool(name="sq", bufs=4) as sq_pool,
            tc.tile_pool(name="psT", bufs=2, space="PSUM") as psT_pool,
            tc.tile_pool(name="psM", bufs=6, space="PSUM") as psM_pool,
            tc.tile_pool(name="outb", bufs=10) as out_pool,
            tc.tile_pool(name="mm", bufs=12) as mm_pool,
        ):
            # identity first (mainline gpsimd ucode), then the scatter
            # library; a throwaway scatter right after eats the hidden
            # ~6us IRAM fetch while DMAs are still in flight
            identity = persist.tile([128, 128], bf16)
            make_identity(nc, identity)
            nc.gpsimd.load_library(library_config.local_scatter)

            labx_sb = persist.tile([128, 2 * RT], i16)
            m_sb = persist.tile([128, 1], f32)

            w_in = persist.tile([128, WT, D], bf16)
            x_in = persist.tile([128, RT, D], bf16)
            w_bf = persist.tile([128, WT, D], bf16)  # normalized W
            wts = persist.tile([128, KC, CL], bf16)  # transposed normalized W
            xts = []
            for t in range(RT):
                xts.append(
                    persist.tile([128, KC, 128], bf16, tag=f"xt{t}", name=f"xt{t}")
                )

            xss = persist.tile([128, RT], f32)
            xnorm = persist.tile([128, RT], f32)
            inv_xnorm = persist.tile([128, RT], f32)
            norm_m = persist.tile([128, RT], f32)
            normm2 = persist.tile([128, 2 * RT], bf16)
            wss = persist.tile([128, WT], f32)
            inv_wnorm = persist.tile([128, WT], f32)
            scr = persist.tile([128, 16], bf16)
            scr_idx = persist.tile([128, 2], i16)

            nc.gpsimd.memset(scr_idx[:], 0)
            nc.gpsimd.local_scatter(
                scr[:],
                identity[:, 0:2],
                scr_idx[:],
                channels=128,
                num_elems=16,
                num_idxs=2,
            )

            nc.vector.memset(w_in[64:, WT - 1, :], 0.0)
            nc.vector.memset(wss[:], 1.0)

            # ---- x + tiny loads on the scalar HWDGE ring ----
            def x_load(t0, t1):
                nc.scalar.dma_start(
                    x_in[:, t0:t1, :],
                    x_ext[128 * t0 : 128 * t1, :].rearrange(
                        "(a p) d -> p a d", p=128
                    ),
                )

            x_load(0, 2)
            nc.scalar.dma_start(labx_sb[:], labx_ext[:])
            nc.scalar.dma_start(m_sb[:], m_ext[:])
            dumm = persist.tile([128, 1], f32)
            nc.scalar.sqrt(dumm[:], wss[:, :1])
            nc.scalar.copy(dumm[:], wss[:, :1])
            x_load(2, 4)

            # ---- W descriptors on the sync HWDGE ring: tiles 0-3 as
            # singles (latency), rest as pairs ----
            def w_load_pair(pr):
                if pr < 7:
                    nc.sync.dma_start(
                        w_in[:, 2 * pr : 2 * pr + 2, :],
                        w_ext[256 * pr : 256 * (pr + 1), :].rearrange(
                            "(a p) d -> p a d", p=128
                        ),
                    )
                else:
                    nc.sync.dma_start(w_in[:, 14, :], w_ext[1792:1920, :])
                    nc.sync.dma_start(w_in[:80, 15, :], w_ext[1920:2000, :])

            for a in range(4):
                nc.sync.dma_start(
                    w_in[:, a, :], w_ext[128 * a : 128 * (a + 1), :]
                )
            for pr in range(2, 8):
                w_load_pair(pr)

            def warmup(n):
                ps = psM_pool.tile([128, 512], f32, tag="psM")
                for _ in range(n):
                    nc.tensor.matmul(
                        ps[:, :128], identity[:], identity[:], start=True, stop=True
                    )

            # ---- W prep ----
            def w_prep(pr):
                a, b = 2 * pr, 2 * pr + 1
                pa = min(128, CL - a * 128)
                pb = min(128, CL - b * 128)
                cs = slice(a, b + 1)
                if pr % 2 == 0:  # batched sumsq on DVE
                    sqw = sq_pool.tile([128, 2, D], bf16, tag="sq")
                    nc.vector.tensor_mul(
                        sqw[:], w_in[:, cs, :], w_in[:, cs, :]
                    )
                    nc.vector.reduce_sum(
                        wss[:, cs].rearrange("p (a b) -> p a b", b=1),
                        sqw[:],
                        axis=mybir.AxisListType.X,
                    )
                else:  # per-tile Square+accum on ACT
                    for c in (a, b):
                        sqc = sq_pool.tile([128, D], bf16, tag="sq")
                        nc.scalar.activation(
                            sqc[:],
                            w_in[:, c, :],
                            mybir.ActivationFunctionType.Square,
                            accum_out=wss[:, c : c + 1],
                        )
                nc.scalar.sqrt(wss[:, cs], wss[:, cs])
                nc.vector.tensor_scalar_max(wss[:, cs], wss[:, cs], 1e-12)
                nc.vector.reciprocal(inv_wnorm[:, cs], wss[:, cs])
                nc.vector.tensor_scalar_mul(
                    w_bf[:pa, a, :], w_in[:pa, a, :], inv_wnorm[:pa, a : a + 1]
                )
                nc.vector.tensor_scalar_mul(
                    w_bf[:pb, b, :], w_in[:pb, b, :], inv_wnorm[:pb, b : b + 1]
                )

            def w_tr(pr):
                a, b = 2 * pr, 2 * pr + 1
                pa = min(128, CL - a * 128)
                pb = min(128, CL - b * 128)
                ps = psT_pool.tile([128, KC, 256], bf16, tag="psT")
                for k in range(KC):
                    nc.tensor.transpose(
                        ps[:, k, :pa],
                        w_bf[:pa, a, k * 128 : (k + 1) * 128],
                        identity[:pa, :pa],
                    )
                    nc.tensor.transpose(
                        ps[:, k, 128 : 128 + pb],
                        w_bf[:pb, b, k * 128 : (k + 1) * 128],
                        identity[:pb, :pb],
                    )
                eng = nc.vector.tensor_copy if pr % 2 == 0 else nc.scalar.copy
                if pr < 7:
                    eng(wts[:, :, 256 * pr : 256 * (pr + 1)], ps[:])
                else:
                    eng(wts[:, :, 1792:1920], ps[:, :, :128])
                    eng(wts[:, :, 1920:2000], ps[:, :, 128:208])

            # ---- X prep ----
            def x_sq2(g):  # row tiles 2g, 2g+1
                a = 2 * g
                cs = slice(a, a + 2)
                if g % 2 == 0:  # ACT per-tile Square+accum
                    for c in (a, a + 1):
                        sqc = sq_pool.tile([128, D], bf16, tag="sq")
                        nc.scalar.activation(
                            sqc[:],
                            x_in[:, c, :],
                            mybir.ActivationFunctionType.Square,
                            accum_out=xss[:, c : c + 1],
                        )
                else:  # DVE batched
                    sqx = sq_pool.tile([128, 2, D], bf16, tag="sq")
                    nc.vector.tensor_mul(
                        sqx[:], x_in[:, cs, :], x_in[:, cs, :]
                    )
                    nc.vector.reduce_sum(
                        xss[:, cs].rearrange("p (a b) -> p a b", b=1),
                        sqx[:],
                        axis=mybir.AxisListType.X,
                    )
                nc.scalar.sqrt(xnorm[:, cs], xss[:, cs])
                nc.vector.tensor_scalar_max(xnorm[:, cs], xnorm[:, cs], 1e-12)
                nc.vector.reciprocal(inv_xnorm[:, cs], xnorm[:, cs])
                nc.vector.tensor_mul(
                    norm_m[:, cs], xnorm[:, cs], m_sb.broadcast_to([128, 2])
                )
                # bf16 [v, v] pairs for the scatter data operand (GPSIMD)
                nc.gpsimd.tensor_copy(
                    normm2[:, 2 * a : 2 * a + 4].rearrange(
                        "p (a b) -> p a b", b=2
                    ),
                    norm_m[:, cs].rearrange("p (a b) -> p a b", b=1).broadcast_to(
                        [128, 2, 2]
                    ),
                )

            def x_tr(t):
                ps = psT_pool.tile([128, KC, 128], bf16, tag="psT")
                for k in range(KC):
                    nc.tensor.transpose(
                        ps[:, k, :],
                        x_in[:, t, k * 128 : (k + 1) * 128],
                        identity[:],
                    )
                if t % 2 == 0:
                    nc.scalar.copy(xts[t][:], ps[:])
                else:
                    nc.vector.tensor_copy(xts[t][:], ps[:])

            masks = {}

            def mask(t):
                mmt = mm_pool.tile([128, CL], bf16, tag="mm")
                nc.gpsimd.local_scatter(
                    mmt[:],
                    normm2[:, 2 * t : 2 * t + 2],
                    labx_sb[:, 2 * t : 2 * t + 2],
                    channels=128,
                    num_elems=CL,
                    num_idxs=2,
                )
                masks[t] = mmt

            # ---- mains: quarter-width single-bank PSUM groups ----
            outs = {}

            def mq(t, h, cc):
                ps = psM_pool.tile([128, 512], f32, tag="psM")
                c0 = h * HW_ + cc * CW
                for k in range(KC):
                    nc.tensor.matmul(
                        ps[:, :CW],
                        xts[t][:, k, :],
                        wts[:, k, c0 : c0 + CW],
                        start=(k == 0),
                        stop=(k == KC - 1),
                    )
                return ps

            def consume(t, h, cc, ps):
                if (t, h) not in outs:
                    cos_h = out_pool.tile([128, HW_], odt, tag="cos")
                    log_h = out_pool.tile([128, HW_], odt, tag="log")
                    outs[(t, h)] = (cos_h, log_h)
                cos_h, log_h = outs[(t, h)]
                sl = slice(cc * CW, (cc + 1) * CW)
                c0 = h * HW_ + cc * CW
                nc.scalar.activation(
                    cos_h[:, sl],
                    ps[:, :CW],
                    mybir.ActivationFunctionType.Copy,
                    scale=inv_xnorm[:, t : t + 1],
                )
                nc.vector.tensor_sub(
                    log_h[:, sl], ps[:, :CW], masks[t][:, c0 : c0 + CW]
                )

            def store(t, h):
                cos_h, log_h = outs.pop((t, h))
                if h == NH - 1:
                    masks.pop(t)
                r0, r1 = t * 128, (t + 1) * 128
                c0 = h * HW_
                if h == NH - 1 and t >= RT - 2:
                    # finer descriptors at the very end: more in flight
                    # while the pipeline drains
                    for q in range(2):
                        ca, cb = c0 + q * CW, c0 + (q + 1) * CW
                        sl = slice(q * CW, (q + 1) * CW)
                        nc.sync.dma_start(cosine_ext[r0:r1, ca:cb], cos_h[:, sl])
                        nc.sync.dma_start(logits_ext[r0:r1, ca:cb], log_h[:, sl])
                else:
                    nc.sync.dma_start(
                        cosine_ext[r0:r1, c0 : c0 + HW_], cos_h[:]
                    )
                    nc.sync.dma_start(
                        logits_ext[r0:r1, c0 : c0 + HW_], log_h[:]
                    )

            # ---- prologue: pairs 0-1 prep, then h=0/cc=0 quarters of
            # tiles 0-3 flow while pairs 2-5 prep rides along ----
            if WARMUP_MM:
                warmup(WARMUP_MM)
            w_prep(0)
            x_tr(0)
            x_tr(1)
            w_tr(0)
            w_prep(1)
            x_sq2(0)
            mask(0)
            mask(1)
            w_tr(1)
            x_tr(2)
            x_tr(3)
            q00 = mq(0, 0, 0)
            consume(0, 0, 0, q00)
            w_prep(2)
            q10 = mq(1, 0, 0)
            consume(1, 0, 0, q10)
            x_sq2(1)
            mask(2)
            mask(3)
            w_tr(2)
            q20 = mq(2, 0, 0)
            consume(2, 0, 0, q20)
            w_prep(3)
            q30 = mq(3, 0, 0)
            consume(3, 0, 0, q30)
            w_tr(3)
            x_load(4, 6)
            ps = mq(0, 0, 1)
            consume(0, 0, 1, ps)
            store(0, 0)
            w_prep(4)
            ps = mq(1, 0, 1)
            consume(1, 0, 1, ps)
            store(1, 0)
            w_tr(4)
            x_load(6, 8)
            ps = mq(2, 0, 1)
            consume(2, 0, 1, ps)
            store(2, 0)
            w_prep(5)
            ps = mq(3, 0, 1)
            consume(3, 0, 1, ps)
            store(3, 0)
            w_tr(5)
            x_tr(4)
            x_tr(5)
            x_sq2(2)
            mask(4)
            mask(5)

            # ---- sweep from t=4: iteration t runs main(t,0) and
            # main(t-CATCH,1); W pairs 6-7 prep during iterations 4-5;
            # row tile t+2 preps during iteration t ----
            for t in range(4, RT + CATCH):
                t0 = t
                t1 = t - CATCH
                if t0 < RT:
                    nt = t0 + 2
                    if t0 - 4 < 2:
                        w_prep(6 + (t0 - 4))
                    ps = mq(t0, 0, 0)
                    consume(t0, 0, 0, ps)
                    ps = mq(t0, 0, 1)
                    if nt < RT:
                        x_tr(nt)
                    consume(t0, 0, 1, ps)
                    store(t0, 0)
                    if t0 - 4 < 2:
                        w_tr(6 + (t0 - 4))
                    if nt < RT:
                        if t0 % 2 == 0 and nt + 2 < RT:
                            x_load(nt + 2, min(nt + 4, RT))
                        if nt % 2 == 1:
                            x_sq2(nt // 2)
                            mask(nt - 1)
                            mask(nt)
                if 0 <= t1 < RT:
                    ps = mq(t1, 1, 0)
                    consume(t1, 1, 0, ps)
                    ps = mq(t1, 1, 1)
                    consume(t1, 1, 1, ps)
                    store(t1, 1)

    nc.finalize()
    return nc


def _in_maps(x, w, lab, mval):
    import ml_dtypes

    bf = ml_dtypes.bfloat16
    maps = []
    lab = np.asarray(lab).astype(np.int64)
    xbf = np.ascontiguousarray(x.astype(bf))
    wbf = np.ascontiguousarray(w.astype(bf))
    for ci in range(8):
        bi, cj = ci // NCL, ci % NCL
        b0, c0 = bi * BL, cj * CL
        ll = (lab[b0 : b0 + BL] - c0).reshape(RT, 128).T  # [128, RT]
        valid = (ll >= 0) & (ll < CL)
        labx = np.full((128, 2 * RT), -2, dtype=np.int16)
        labx[:, 0::2] = np.where(valid, ll, -1).astype(np.int16)
        maps.append(
            {
                "x": xbf[b0 : b0 + BL],
                "w": wbf[c0 : c0 + CL],
                "labx": np.ascontiguousarray(labx),
                "mvec": np.full((128, 1), mval, dtype=np.float32),
            }
        )
    return maps


def kernel(input, label, weight, s, m):
    from concourse.bass_utils import run_bass_kernel_spmd

    if "nc" not in _CACHE:
        _CACHE["nc"] = _build()
    nc = _CACHE["nc"]

    x = np.ascontiguousarray(np.asarray(input, dtype=np.float32))
    w = np.ascontiguousarray(np.asarray(weight, dtype=np.float32))
    lab = np.asarray(label)
    mval = float(np.asarray(m))

    res = run_bass_kernel_spmd(nc, _in_maps(x, w, lab, mval), core_ids=list(range(8)))

    logits = np.empty((B, C), dtype=np.float32)
    cosine = np.empty((B, C), dtype=np.float32)
    for ci in range(8):
        bi, cj = ci // NCL, ci % NCL
        b0, c0 = bi * BL, cj * CL
        logits[b0 : b0 + BL, c0 : c0 + CL] = np.asarray(
            res.results[ci]["logits"], dtype=np.float32
        )
        cosine[b0 : b0 + BL, c0 : c0 + CL] = np.asarray(
            res.results[ci]["cosine"], dtype=np.float32
        )
    return logits, cosine


# revision 27
# speedup vs baseline: 1.0108x; 1.0108x over previous
"""AMSoftmax (norm-free branch) Trainium2 kernel, 8 NeuronCores.

Reference computes, for input x [B,D], label [B], weight [C,D], scalars s,m:
    norm   = ||x||_2 per row                       [B,1]
    cosine = (x/max(norm,eps)) @ (w/max(||w||,eps)).T   [B,C]
    logits = norm * (cosine - m*onehot(label))     [B,C]
    returns (logits, cosine)

Key identity: norm * cosine == x @ w_hat.T exactly, so per output element:
    raw    = x @ w_hat.T          (PSUM, f32)
    cosine = raw * (1/norm)       (per-row scale, ACT)
    logits = raw - norm*m*onehot  (DVE sub against a sparse mask)

Sharding: 2-way over batch x 4-way over classes (8 cores, no collectives;
outputs are disjoint tiles concatenated on host). Per core: x [2048,512],
w_hat [2000,512], outputs [2048,2000] each, stored as bf16.

v6:
- x and w shipped bf16 (host dtype prep): input DMA is 4.2MB, no casts.
- W pairs: sumsq (alternating DVE-batched / ACT+accum), scale-cast on
  DVE, fast is_transpose into bf16 PSUM, one merged copy per pair.
- Quarter-width single-bank PSUM groups, 6-deep psM pool.
- Identity built before the GPSIMD library load; a dummy scatter right
  after the load eats the ~6us hidden IRAM fetch inside the DMA shadow.
- W pair prep for pairs 2-7 rides iterations 0-5 (CATCH=6 so h=1 starts
  after all W is ready); stores on the sync ring strictly after inputs.
- PE warm-up bursts keep the HAM clock gate open through the prologue.
"""

import os
import sys

sys.path.insert(0, "/opt/trn_rl_repo")

import numpy as np

B, D, C = 4096, 512, 8000
NB, NCL = 2, 4  # batch x class core grid
BL, CL = B // NB, C // NCL  # 2048, 2000 per core
RT = BL // 128  # 16 row tiles
KC = D // 128  # 4 contraction chunks
CW = 500  # matmul free-dim chunk (PSUM bank holds 512 f32)
HW_ = 2 * CW  # 1000 columns per half
NH = CL // HW_  # 2 column halves per row tile

OUT_BF16 = os.environ.get("AMS_OUT", "bf16") == "bf16"
WARMUP_MM = int(os.environ.get("AMS_WARMUP", "20"))
WARMUP2_MM = int(os.environ.get("AMS_WARMUP2", "40"))
CATCH = int(os.environ.get("AMS_CATCH", "6"))  # h=1 catch-up offset

_CACHE = {}


def _build():
    import concourse.mybir as mybir
    import concourse.tile as tile
    from concourse import bacc, library_config
    from concourse.masks import make_identity

    f32 = mybir.dt.float32
    i16 = mybir.dt.int16
    bf16 = mybir.dt.bfloat16
    odt = bf16 if OUT_BF16 else f32

    nc = bacc.Bacc()
    x_ext = nc.declare_dram_parameter("x", [BL, D], bf16, isOutput=False)
    w_ext = nc.declare_dram_parameter("w", [CL, D], bf16, isOutput=False)
    labx_ext = nc.declare_dram_parameter("labx", [128, 2 * RT], i16, isOutput=False)
    m_ext = nc.declare_dram_parameter("mvec", [128, 1], f32, isOutput=False)
    logits_ext = nc.declare_dram_parameter("logits", [BL, CL], odt, isOutput=True)
    cosine_ext = nc.declare_dram_parameter("cosine", [BL, CL], odt, isOutput=True)

    WT = (CL + 127) // 128  # 16 w row tiles (last one 80 partitions)

    with tile.TileContext(nc) as tc:
        with (
            tc.tile_pool(name="persist", bufs=1) as persist,
            tc.tile_pool(name="sq", bufs=4) as sq_pool,
            tc.tile_pool(name="psT", bufs=2, space="PSUM") as psT_pool,
            tc.tile_pool(name="psM", bufs=3, space="PSUM") as psM_pool,
            tc.tile_pool(name="outb", bufs=10) as out_pool,
            tc.tile_pool(name="mm", bufs=12) as mm_pool,
        ):
            # identity first (mainline gpsimd ucode), then the scatter
            # library; a throwaway scatter right after eats the hidden
            # ~6us IRAM fetch while DMAs are still in flight
            identity = persist.tile([128, 128], bf16)
            make_identity(nc, identity)
            nc.gpsimd.load_library(library_config.local_scatter)

            labx_sb = persist.tile([128, 2 * RT], i16)
            m_sb = persist.tile([128, 1], f32)

            w_in = persist.tile([128, WT, D], bf16)
            x_in = persist.tile([128, RT, D], bf16)
            w_bf = persist.tile([128, WT, D], bf16)  # normalized W
            wts = persist.tile([128, KC, CL], bf16)  # transposed normalized W
            xts = []
            for t in range(RT):
                xts.append(
                    persist.tile([128, KC, 128], bf16, tag=f"xt{t}", name=f"xt{t}")
                )

            xss = persist.tile([128, RT], f32)
            xnorm = persist.tile([128, RT], f32)
            inv_xnorm = persist.tile([128, RT], f32)
            norm_m = persist.tile([128, RT], f32)
            normm2 = persist.tile([128, 2 * RT], bf16)
            wss = persist.tile([128, WT], f32)
            inv_wnorm = persist.tile([128, WT], f32)
            scr = persist.tile([128, 16], bf16)
            scr_idx = persist.tile([128, 2], i16)

            nc.gpsimd.memset(scr_idx[:], 0)
            nc.gpsimd.local_scatter(
                scr[:],
                identity[:, 0:2],
                scr_idx[:],
                channels=128,
                num_elems=16,
                num_idxs=2,
            )

            nc.vector.memset(w_in[64:, WT - 1, :], 0.0)
            nc.vector.memset(wss[:], 1.0)

            # ---- x + tiny loads on the scalar HWDGE ring ----
            def x_load(t0, t1):
                nc.scalar.dma_start(
                    x_in[:, t0:t1, :],
                    x_ext[128 * t0 : 128 * t1, :].rearrange(
                        "(a p) d -> p a d", p=128
                    ),
                )

            x_load(0, 2)
            nc.scalar.dma_start(labx_sb[:], labx_ext[:])
            nc.scalar.dma_start(m_sb[:], m_ext[:])
            dumm = persist.tile([128, 1], f32)
            nc.scalar.sqrt(dumm[:], wss[:, :1])
            nc.scalar.copy(dumm[:], wss[:, :1])
            x_load(2, 4)

            # ---- W descriptors on the sync HWDGE ring: tiles 0-3 as
            # singles (latency), rest as pairs ----
            def w_load_pair(pr):
                if pr < 7:
                    nc.sync.dma_start(
                        w_in[:, 2 * pr : 2 * pr + 2, :],
                        w_ext[256 * pr : 256 * (pr + 1), :].rearrange(
                            "(a p) d -> p a d", p=128
                        ),
                    )
                else:
                    nc.sync.dma_start(w_in[:, 14, :], w_ext[1792:1920, :])
                    nc.sync.dma_start(w_in[:80, 15, :], w_ext[1920:2000, :])

            for a in range(4):
                nc.sync.dma_start(
                    w_in[:, a, :], w_ext[128 * a : 128 * (a + 1), :]
                )
            for pr in range(2, 8):
                w_load_pair(pr)

            def warmup(n):
                ps = psM_pool.tile([128, 2, 512], f32, tag="psM")
                for _ in range(n):
                    nc.tensor.matmul(
                        ps[:, 0, :128], identity[:], identity[:], start=True, stop=True
                    )

            # ---- W prep ----
            def w_prep(pr):
                a, b = 2 * pr, 2 * pr + 1
                pa = min(128, CL - a * 128)
                pb = min(128, CL - b * 128)
                cs = slice(a, b + 1)
                if pr % 2 == 0:  # batched sumsq on DVE
                    sqw = sq_pool.tile([128, 2, D], bf16, tag="sq")
                    nc.vector.tensor_mul(
                        sqw[:], w_in[:, cs, :], w_in[:, cs, :]
                    )
                    nc.vector.reduce_sum(
                        wss[:, cs].rearrange("p (a b) -> p a b", b=1),
                        sqw[:],
                        axis=mybir.AxisListType.X,
                    )
                else:  # per-tile Square+accum on ACT
                    for c in (a, b):
                        sqc = sq_pool.tile([128, D], bf16, tag="sq")
                        nc.scalar.activation(
                            sqc[:],
                            w_in[:, c, :],
                            mybir.ActivationFunctionType.Square,
                            accum_out=wss[:, c : c + 1],
                        )
                nc.scalar.sqrt(wss[:, cs], wss[:, cs])
                nc.vector.tensor_scalar_max(wss[:, cs], wss[:, cs], 1e-12)
                nc.vector.reciprocal(inv_wnorm[:, cs], wss[:, cs])
                nc.vector.tensor_scalar_mul(
                    w_bf[:pa, a, :], w_in[:pa, a, :], inv_wnorm[:pa, a : a + 1]
                )
                nc.vector.tensor_scalar_mul(
                    w_bf[:pb, b, :], w_in[:pb, b, :], inv_wnorm[:pb, b : b + 1]
                )

            def w_tr(pr):
                a, b = 2 * pr, 2 * pr + 1
                pa = min(128, CL - a * 128)
                pb = min(128, CL - b * 128)
                ps = psT_pool.tile([128, KC, 256], bf16, tag="psT")
                for k in range(KC):
                    nc.tensor.transpose(
                        ps[:, k, :pa],
                        w_bf[:pa, a, k * 128 : (k + 1) * 128],
                        identity[:pa, :pa],
                    )
                    nc.tensor.transpose(
                        ps[:, k, 128 : 128 + pb],
                        w_bf[:pb, b, k * 128 : (k + 1) * 128],
                        identity[:pb, :pb],
                    )
                eng = nc.vector.tensor_copy if pr % 2 == 0 else nc.scalar.copy
                if pr < 7:
                    eng(wts[:, :, 256 * pr : 256 * (pr + 1)], ps[:])
                else:
                    eng(wts[:, :, 1792:1920], ps[:, :, :128])
                    eng(wts[:, :, 1920:2000], ps[:, :, 128:208])

            # ---- X prep ----
            def x_sq(c):  # one row tile sumsq on ACT
                sqc = sq_pool.tile([128, D], bf16, tag="sq")
                nc.scalar.activation(
                    sqc[:],
                    x_in[:, c, :],
                    mybir.ActivationFunctionType.Square,
                    accum_out=xss[:, c : c + 1],
                )

            def x_norms2(g):  # norm chain for row tiles 2g, 2g+1
                a = 2 * g
                cs = slice(a, a + 2)
                nc.scalar.sqrt(xnorm[:, cs], xss[:, cs])
                nc.vector.tensor_scalar_max(xnorm[:, cs], xnorm[:, cs], 1e-12)
                nc.vector.reciprocal(inv_xnorm[:, cs], xnorm[:, cs])
                nc.vector.tensor_mul(
                    norm_m[:, cs], xnorm[:, cs], m_sb.broadcast_to([128, 2])
                )
                # bf16 [v, v] pairs for the scatter data operand (GPSIMD)
                nc.gpsimd.tensor_copy(
                    normm2[:, 2 * a : 2 * a + 4].rearrange(
                        "p (a b) -> p a b", b=2
                    ),
                    norm_m[:, cs].rearrange("p (a b) -> p a b", b=1).broadcast_to(
                        [128, 2, 2]
                    ),
                )

            def x_tr(t):
                ps = psT_pool.tile([128, KC, 128], bf16, tag="psT")
                for k in range(KC):
                    nc.tensor.transpose(
                        ps[:, k, :],
                        x_in[:, t, k * 128 : (k + 1) * 128],
                        identity[:],
                    )
                nc.vector.tensor_copy(xts[t][:], ps[:])

            masks = {}

            def mask(t):
                mmt = mm_pool.tile([128, CL], bf16, tag="mm")
                nc.gpsimd.local_scatter(
                    mmt[:],
                    normm2[:, 2 * t : 2 * t + 2],
                    labx_sb[:, 2 * t : 2 * t + 2],
                    channels=128,
                    num_elems=CL,
                    num_idxs=2,
                )
                masks[t] = mmt

            # ---- mains: quarter-width single-bank PSUM groups ----
            outs = {}

            def main_h(t, h):
                # cc-outer: the first 4 matmuls need only W pairs 0-1 of
                # this half's first quarter
                ps = psM_pool.tile([128, 2, 512], f32, tag="psM")
                for cc in range(2):
                    c0 = h * HW_ + cc * CW
                    for k in range(KC):
                        nc.tensor.matmul(
                            ps[:, cc, :CW],
                            xts[t][:, k, :],
                            wts[:, k, c0 : c0 + CW],
                            start=(k == 0),
                            stop=(k == KC - 1),
                        )
                ps3 = ps[:, :, :CW]
                cos_h = out_pool.tile([128, HW_], odt, tag="cos")
                log_h = out_pool.tile([128, HW_], odt, tag="log")
                outs[(t, h)] = (cos_h, log_h)
                c0 = h * HW_
                nc.scalar.activation(
                    cos_h[:].rearrange("p (a b) -> p a b", a=2),
                    ps3,
                    mybir.ActivationFunctionType.Copy,
                    scale=inv_xnorm[:, t : t + 1],
                )
                nc.vector.tensor_sub(
                    log_h[:].rearrange("p (a b) -> p a b", a=2),
                    ps3,
                    masks[t][:, c0 : c0 + HW_].rearrange("p (a b) -> p a b", a=2),
                )

            def store(t, h):
                cos_h, log_h = outs.pop((t, h))
                if h == NH - 1:
                    masks.pop(t)
                r0, r1 = t * 128, (t + 1) * 128
                c0 = h * HW_
                if h == NH - 1 and t >= RT - 2:
                    # finer descriptors at the very end: more in flight
                    # while the pipeline drains
                    for q in range(2):
                        ca, cb = c0 + q * CW, c0 + (q + 1) * CW
                        sl = slice(q * CW, (q + 1) * CW)
                        nc.sync.dma_start(cosine_ext[r0:r1, ca:cb], cos_h[:, sl])
                        nc.sync.dma_start(logits_ext[r0:r1, ca:cb], log_h[:, sl])
                else:
                    nc.sync.dma_start(
                        cosine_ext[r0:r1, c0 : c0 + HW_], cos_h[:]
                    )
                    nc.sync.dma_start(
                        logits_ext[r0:r1, c0 : c0 + HW_], log_h[:]
                    )

            # ---- prologue: pairs 0-1 prep, then h=0/cc=0 quarters of
            # tiles 0-3 flow while pairs 2-5 prep rides along ----
            if WARMUP_MM:
                warmup(WARMUP_MM)
            w_prep(0)
            x_tr(0)
            x_tr(1)
            w_tr(0)
            w_prep(1)
            x_sq(0)
            x_sq(1)
            x_norms2(0)
            mask(0)
            mask(1)
            w_tr(1)
            x_tr(2)
            x_tr(3)
            w_prep(2)
            w_tr(2)
            w_prep(3)
            w_tr(3)
            main_h(0, 0)
            x_sq(2)
            w_prep(4)
            main_h(1, 0)
            store(0, 0)
            x_sq(3)
            x_norms2(1)
            mask(2)
            mask(3)
            w_tr(4)
            x_load(4, 6)
            main_h(2, 0)
            store(1, 0)
            w_prep(5)
            main_h(3, 0)
            store(2, 0)
            w_tr(5)
            x_load(6, 8)
            x_tr(4)
            x_tr(5)
            x_sq(4)
            x_sq(5)
            x_norms2(2)
            mask(4)
            mask(5)
            store(3, 0)

            # ---- sweep from t=4: iteration t runs main(t,0) and
            # main(t-CATCH,1); W pairs 6-7 prep during iterations 4-5;
            # row tile t+2 preps during iteration t ----
            for t in range(4, RT + CATCH):
                t0 = t
                t1 = t - CATCH
                if t0 < RT:
                    nt = t0 + 2
                    if t0 - 4 < 2:
                        w_prep(6 + (t0 - 4))
                    main_h(t0, 0)
                    if nt < RT:
                        x_tr(nt)
                    store(t0, 0)
                    if t0 - 4 < 2:
                        w_tr(6 + (t0 - 4))
                    if nt < RT:
                        if t0 % 2 == 0 and nt + 2 < RT:
                            x_load(nt + 2, min(nt + 4, RT))
                        x_sq(nt)
                        if nt % 2 == 1:
                            x_norms2(nt // 2)
                            mask(nt - 1)
                            mask(nt)
                if 0 <= t1 < RT:
                    main_h(t1, 1)
                    store(t1, 1)

    nc.finalize()
    return nc


def _in_maps(x, w, lab, mval):
    import ml_dtypes

    bf = ml_dtypes.bfloat16
    maps = []
    lab = np.asarray(lab).astype(np.int64)
    xbf = np.ascontiguousarray(x.astype(bf))
    wbf = np.ascontiguousarray(w.astype(bf))
    for ci in range(8):
        bi, cj = ci // NCL, ci % NCL
        b0, c0 = bi * BL, cj * CL
        ll = (lab[b0 : b0 + BL] - c0).reshape(RT, 128).T  # [128, RT]
        valid = (ll >= 0) & (ll < CL)
        labx = np.full((128, 2 * RT), -2, dtype=np.int16)
        labx[:, 0::2] = np.where(valid, ll, -1).astype(np.int16)
        maps.append(
            {
                "x": xbf[b0 : b0 + BL],
                "w": wbf[c0 : c0 + CL],
                "labx": np.ascontiguousarray(labx),
                "mvec": np.full((128, 1), mval, dtype=np.float32),
            }
        )
    return maps


def kernel(input, label, weight, s, m):
    from concourse.bass_utils import run_bass_kernel_spmd

    if "nc" not in _CACHE:
        _CACHE["nc"] = _build()
    nc = _CACHE["nc"]

    x = np.ascontiguousarray(np.asarray(input, dtype=np.float32))
    w = np.ascontiguousarray(np.asarray(weight, dtype=np.float32))
    lab = np.asarray(label)
    mval = float(np.asarray(m))

    res = run_bass_kernel_spmd(nc, _in_maps(x, w, lab, mval), core_ids=list(range(8)))

    logits = np.empty((B, C), dtype=np.float32)
    cosine = np.empty((B, C), dtype=np.float32)
    for ci in range(8):
        bi, cj = ci // NCL, ci % NCL
        b0, c0 = bi * BL, cj * CL
        logits[b0 : b0 + BL, c0 : c0 + CL] = np.asarray(
            res.results[ci]["logits"], dtype=np.float32
        )
        cosine[b0 : b0 + BL, c0 : c0 + CL] = np.asarray(
            res.results[ci]["cosine"], dtype=np.float32
        )
    return logits, cosine


# revision 28
# speedup vs baseline: 1.1999x; 1.1870x over previous
"""AMSoftmax (norm-free branch) Trainium2 kernel, 8 NeuronCores.

Reference computes, for input x [B,D], label [B], weight [C,D], scalars s,m:
    norm   = ||x||_2 per row                       [B,1]
    cosine = (x/max(norm,eps)) @ (w/max(||w||,eps)).T   [B,C]
    logits = norm * (cosine - m*onehot(label))     [B,C]
    returns (logits, cosine)

Key identity: norm * cosine == x @ w_hat.T exactly, so per output element:
    raw    = x @ w_hat.T          (PSUM, f32)
    cosine = raw * (1/norm)       (per-row scale, ACT)
    logits = raw - norm*m*onehot  (DVE sub against a sparse mask)

Sharding: 2-way over batch x 4-way over classes (8 cores, no collectives;
outputs are disjoint tiles concatenated on host). Per core: x [2048,512],
w_hat [2000,512], outputs [2048,2000] each, stored as bf16.

v6:
- x and w shipped bf16 (host dtype prep): input DMA is 4.2MB, no casts.
- W pairs: sumsq (alternating DVE-batched / ACT+accum), scale-cast on
  DVE, fast is_transpose into bf16 PSUM, one merged copy per pair.
- Quarter-width single-bank PSUM groups, 6-deep psM pool.
- Identity built before the GPSIMD library load; a dummy scatter right
  after the load eats the ~6us hidden IRAM fetch inside the DMA shadow.
- W pair prep for pairs 2-7 rides iterations 0-5 (CATCH=6 so h=1 starts
  after all W is ready); stores on the sync ring strictly after inputs.
- PE warm-up bursts keep the HAM clock gate open through the prologue.
"""

import os
import sys

sys.path.insert(0, "/opt/trn_rl_repo")

import numpy as np

B, D, C = 4096, 512, 8000
NB, NCL = 2, 4  # batch x class core grid
BL, CL = B // NB, C // NCL  # 2048, 2000 per core
RT = BL // 128  # 16 row tiles
KC = D // 128  # 4 contraction chunks
CW = 500  # matmul free-dim chunk (PSUM bank holds 512 f32)
HW_ = 2 * CW  # 1000 columns per half
NH = CL // HW_  # 2 column halves per row tile

OUT_BF16 = os.environ.get("AMS_OUT", "bf16") == "bf16"
WARMUP_MM = int(os.environ.get("AMS_WARMUP", "20"))
WARMUP2_MM = int(os.environ.get("AMS_WARMUP2", "40"))
CATCH = int(os.environ.get("AMS_CATCH", "6"))  # h=1 catch-up offset

_CACHE = {}


def _build():
    import concourse.mybir as mybir
    import concourse.tile as tile
    from concourse import bacc, library_config
    from concourse.masks import make_identity

    f32 = mybir.dt.float32
    i16 = mybir.dt.int16
    bf16 = mybir.dt.bfloat16
    odt = bf16 if OUT_BF16 else f32

    nc = bacc.Bacc()
    x_ext = nc.declare_dram_parameter("x", [BL, D], bf16, isOutput=False)
    w_ext = nc.declare_dram_parameter("w", [CL, D], bf16, isOutput=False)
    labx_ext = nc.declare_dram_parameter("labx", [128, 2 * RT], i16, isOutput=False)
    m_ext = nc.declare_dram_parameter("mvec", [128, 1], f32, isOutput=False)
    logits_ext = nc.declare_dram_parameter("logits", [BL, CL], odt, isOutput=True)
    cosine_ext = nc.declare_dram_parameter("cosine", [BL, CL], odt, isOutput=True)

    WT = (CL + 127) // 128  # 16 w row tiles (last one 80 partitions)

    with tile.TileContext(nc) as tc:
        with (
            tc.tile_pool(name="persist", bufs=1) as persist,
            tc.tile_pool(name="sq", bufs=4) as sq_pool,
            tc.tile_pool(name="psT", bufs=2, space="PSUM") as psT_pool,
            tc.tile_pool(name="psM", bufs=3, space="PSUM") as psM_pool,
            tc.tile_pool(name="outb", bufs=16) as out_pool,
            tc.tile_pool(name="mm", bufs=12) as mm_pool,
        ):
            # identity first (mainline gpsimd ucode), then the scatter
            # library; a throwaway scatter right after eats the hidden
            # ~6us IRAM fetch while DMAs are still in flight
            identity = persist.tile([128, 128], bf16)
            make_identity(nc, identity)
            nc.gpsimd.load_library(library_config.local_scatter)

            labx_sb = persist.tile([128, 2 * RT], i16)
            m_sb = persist.tile([128, 1], f32)

            w_in = persist.tile([128, WT, D], bf16)
            x_in = persist.tile([128, RT, D], bf16)
            w_bf = persist.tile([128, WT, D], bf16)  # normalized W
            wts = persist.tile([128, KC, CL], bf16)  # transposed normalized W
            xts = []
            for t in range(RT):
                xts.append(
                    persist.tile([128, KC, 128], bf16, tag=f"xt{t}", name=f"xt{t}")
                )

            xss = persist.tile([128, RT], f32)
            xnorm = persist.tile([128, RT], f32)
            inv_xnorm = persist.tile([128, RT], f32)
            norm_m = persist.tile([128, RT], f32)
            normm2 = persist.tile([128, 2 * RT], bf16)
            wss = persist.tile([128, WT], f32)
            inv_wnorm = persist.tile([128, WT], f32)
            scr = persist.tile([128, 16], bf16)
            scr_idx = persist.tile([128, 2], i16)

            nc.gpsimd.memset(scr_idx[:], 0)
            nc.gpsimd.local_scatter(
                scr[:],
                identity[:, 0:2],
                scr_idx[:],
                channels=128,
                num_elems=16,
                num_idxs=2,
            )

            nc.vector.memset(w_in[64:, WT - 1, :], 0.0)
            nc.vector.memset(wss[:], 1.0)

            # ---- x + tiny loads on the scalar HWDGE ring ----
            def x_load(t0, t1):
                nc.scalar.dma_start(
                    x_in[:, t0:t1, :],
                    x_ext[128 * t0 : 128 * t1, :].rearrange(
                        "(a p) d -> p a d", p=128
                    ),
                )

            x_load(0, 2)
            nc.scalar.dma_start(labx_sb[:], labx_ext[:])
            nc.scalar.dma_start(m_sb[:], m_ext[:])
            dumm = persist.tile([128, 1], f32)
            nc.scalar.sqrt(dumm[:], wss[:, :1])
            nc.scalar.copy(dumm[:], wss[:, :1])
            x_load(2, 4)

            # ---- W descriptors on the sync HWDGE ring: tiles 0-3 as
            # singles (latency), rest as pairs ----
            def w_load_pair(pr):
                if pr < 7:
                    nc.sync.dma_start(
                        w_in[:, 2 * pr : 2 * pr + 2, :],
                        w_ext[256 * pr : 256 * (pr + 1), :].rearrange(
                            "(a p) d -> p a d", p=128
                        ),
                    )
                else:
                    nc.sync.dma_start(w_in[:, 14, :], w_ext[1792:1920, :])
                    nc.sync.dma_start(w_in[:80, 15, :], w_ext[1920:2000, :])

            for a in range(4):
                nc.sync.dma_start(
                    w_in[:, a, :], w_ext[128 * a : 128 * (a + 1), :]
                )
            for pr in range(2, 8):
                w_load_pair(pr)

            def warmup(n):
                ps = psM_pool.tile([128, 2, 512], f32, tag="psM")
                for _ in range(n):
                    nc.tensor.matmul(
                        ps[:, 0, :128], identity[:], identity[:], start=True, stop=True
                    )

            # ---- W prep ----
            def w_prep(pr):
                a, b = 2 * pr, 2 * pr + 1
                pa = min(128, CL - a * 128)
                pb = min(128, CL - b * 128)
                cs = slice(a, b + 1)
                if pr % 2 == 0:  # batched sumsq on DVE
                    sqw = sq_pool.tile([128, 2, D], bf16, tag="sq")
                    nc.vector.tensor_mul(
                        sqw[:], w_in[:, cs, :], w_in[:, cs, :]
                    )
                    nc.vector.reduce_sum(
                        wss[:, cs].rearrange("p (a b) -> p a b", b=1),
                        sqw[:],
                        axis=mybir.AxisListType.X,
                    )
                else:  # per-tile Square+accum on ACT
                    for c in (a, b):
                        sqc = sq_pool.tile([128, D], bf16, tag="sq")
                        nc.scalar.activation(
                            sqc[:],
                            w_in[:, c, :],
                            mybir.ActivationFunctionType.Square,
                            accum_out=wss[:, c : c + 1],
                        )
                nc.scalar.sqrt(wss[:, cs], wss[:, cs])
                nc.vector.tensor_scalar_max(wss[:, cs], wss[:, cs], 1e-12)
                nc.vector.reciprocal(inv_wnorm[:, cs], wss[:, cs])
                nc.vector.tensor_scalar_mul(
                    w_bf[:pa, a, :], w_in[:pa, a, :], inv_wnorm[:pa, a : a + 1]
                )
                nc.vector.tensor_scalar_mul(
                    w_bf[:pb, b, :], w_in[:pb, b, :], inv_wnorm[:pb, b : b + 1]
                )

            def w_tr(pr):
                a, b = 2 * pr, 2 * pr + 1
                pa = min(128, CL - a * 128)
                pb = min(128, CL - b * 128)
                ps = psT_pool.tile([128, KC, 256], bf16, tag="psT")
                for k in range(KC):
                    nc.tensor.transpose(
                        ps[:, k, :pa],
                        w_bf[:pa, a, k * 128 : (k + 1) * 128],
                        identity[:pa, :pa],
                    )
                    nc.tensor.transpose(
                        ps[:, k, 128 : 128 + pb],
                        w_bf[:pb, b, k * 128 : (k + 1) * 128],
                        identity[:pb, :pb],
                    )
                eng = nc.vector.tensor_copy if pr % 2 == 0 else nc.scalar.copy
                if pr < 7:
                    eng(wts[:, :, 256 * pr : 256 * (pr + 1)], ps[:])
                else:
                    eng(wts[:, :, 1792:1920], ps[:, :, :128])
                    eng(wts[:, :, 1920:2000], ps[:, :, 128:208])

            # ---- X prep ----
            def x_sq(c):  # one row tile sumsq on ACT
                sqc = sq_pool.tile([128, D], bf16, tag="sq")
                nc.scalar.activation(
                    sqc[:],
                    x_in[:, c, :],
                    mybir.ActivationFunctionType.Square,
                    accum_out=xss[:, c : c + 1],
                )

            def x_norms2(g):  # norm chain for row tiles 2g, 2g+1
                a = 2 * g
                cs = slice(a, a + 2)
                nc.scalar.sqrt(xnorm[:, cs], xss[:, cs])
                nc.vector.tensor_scalar_max(xnorm[:, cs], xnorm[:, cs], 1e-12)
                nc.vector.reciprocal(inv_xnorm[:, cs], xnorm[:, cs])
                nc.vector.tensor_mul(
                    norm_m[:, cs], xnorm[:, cs], m_sb.broadcast_to([128, 2])
                )
                # bf16 [v, v] pairs for the scatter data operand (GPSIMD)
                nc.gpsimd.tensor_copy(
                    normm2[:, 2 * a : 2 * a + 4].rearrange(
                        "p (a b) -> p a b", b=2
                    ),
                    norm_m[:, cs].rearrange("p (a b) -> p a b", b=1).broadcast_to(
                        [128, 2, 2]
                    ),
                )

            def x_tr(t):
                ps = psT_pool.tile([128, KC, 128], bf16, tag="psT")
                for k in range(KC):
                    nc.tensor.transpose(
                        ps[:, k, :],
                        x_in[:, t, k * 128 : (k + 1) * 128],
                        identity[:],
                    )
                nc.vector.tensor_copy(xts[t][:], ps[:])

            masks = {}

            def mask(t):
                mmt = mm_pool.tile([128, CL], bf16, tag="mm")
                nc.gpsimd.local_scatter(
                    mmt[:],
                    normm2[:, 2 * t : 2 * t + 2],
                    labx_sb[:, 2 * t : 2 * t + 2],
                    channels=128,
                    num_elems=CL,
                    num_idxs=2,
                )
                masks[t] = mmt

            # ---- mains: quarter-width single-bank PSUM groups ----
            outs = {}

            def main_h(t, h):
                # cc-outer: the first 4 matmuls need only W pairs 0-1 of
                # this half's first quarter
                ps = psM_pool.tile([128, 2, 512], f32, tag="psM")
                for cc in range(2):
                    c0 = h * HW_ + cc * CW
                    for k in range(KC):
                        nc.tensor.matmul(
                            ps[:, cc, :CW],
                            xts[t][:, k, :],
                            wts[:, k, c0 : c0 + CW],
                            start=(k == 0),
                            stop=(k == KC - 1),
                        )
                ps3 = ps[:, :, :CW]
                cos_h = out_pool.tile([128, HW_], odt, tag="cos")
                log_h = out_pool.tile([128, HW_], odt, tag="log")
                outs[(t, h)] = (cos_h, log_h)
                c0 = h * HW_
                nc.scalar.activation(
                    cos_h[:].rearrange("p (a b) -> p a b", a=2),
                    ps3,
                    mybir.ActivationFunctionType.Copy,
                    scale=inv_xnorm[:, t : t + 1],
                )
                nc.vector.tensor_sub(
                    log_h[:].rearrange("p (a b) -> p a b", a=2),
                    ps3,
                    masks[t][:, c0 : c0 + HW_].rearrange("p (a b) -> p a b", a=2),
                )

            def store(t, h):
                cos_h, log_h = outs.pop((t, h))
                if h == NH - 1:
                    masks.pop(t)
                r0, r1 = t * 128, (t + 1) * 128
                c0 = h * HW_
                if h == NH - 1 and t >= RT - 2:
                    # finer descriptors at the very end: more in flight
                    # while the pipeline drains
                    for q in range(2):
                        ca, cb = c0 + q * CW, c0 + (q + 1) * CW
                        sl = slice(q * CW, (q + 1) * CW)
                        nc.sync.dma_start(cosine_ext[r0:r1, ca:cb], cos_h[:, sl])
                        nc.sync.dma_start(logits_ext[r0:r1, ca:cb], log_h[:, sl])
                else:
                    nc.sync.dma_start(
                        cosine_ext[r0:r1, c0 : c0 + HW_], cos_h[:]
                    )
                    nc.sync.dma_start(
                        logits_ext[r0:r1, c0 : c0 + HW_], log_h[:]
                    )

            # ---- prologue: pairs 0-1 prep, then h=0/cc=0 quarters of
            # tiles 0-3 flow while pairs 2-5 prep rides along ----
            if WARMUP_MM:
                warmup(WARMUP_MM)
            w_prep(0)
            x_tr(0)
            x_tr(1)
            w_tr(0)
            w_prep(1)
            x_sq(0)
            x_sq(1)
            x_norms2(0)
            mask(0)
            mask(1)
            w_tr(1)
            x_tr(2)
            x_tr(3)
            w_prep(2)
            w_tr(2)
            w_prep(3)
            w_tr(3)
            main_h(0, 0)
            x_sq(2)
            w_prep(4)
            main_h(1, 0)
            store(0, 0)
            x_sq(3)
            x_norms2(1)
            mask(2)
            mask(3)
            w_tr(4)
            x_load(4, 6)
            main_h(2, 0)
            store(1, 0)
            w_prep(5)
            main_h(3, 0)
            store(2, 0)
            w_tr(5)
            x_load(6, 8)
            x_tr(4)
            x_tr(5)
            x_sq(4)
            x_sq(5)
            x_norms2(2)
            mask(4)
            mask(5)
            store(3, 0)

            # ---- sweep from t=4: iteration t runs main(t,0) and
            # main(t-CATCH,1); W pairs 6-7 prep during iterations 4-5;
            # row tile t+2 preps during iteration t ----
            for t in range(4, RT + CATCH):
                t0 = t
                t1 = t - CATCH
                if t0 < RT:
                    nt = t0 + 2
                    if t0 - 4 < 2:
                        w_prep(6 + (t0 - 4))
                    main_h(t0, 0)
                    if nt < RT:
                        x_tr(nt)
                    store(t0, 0)
                    if t0 - 4 < 2:
                        w_tr(6 + (t0 - 4))
                    if nt < RT:
                        if t0 % 2 == 0 and nt + 2 < RT:
                            x_load(nt + 2, min(nt + 4, RT))
                        x_sq(nt)
                        if nt % 2 == 1:
                            x_norms2(nt // 2)
                            mask(nt - 1)
                            mask(nt)
                if 0 <= t1 < RT:
                    main_h(t1, 1)
                    store(t1, 1)

    nc.finalize()
    return nc


def _in_maps(x, w, lab, mval):
    import ml_dtypes

    bf = ml_dtypes.bfloat16
    maps = []
    lab = np.asarray(lab).astype(np.int64)
    xbf = np.ascontiguousarray(x.astype(bf))
    wbf = np.ascontiguousarray(w.astype(bf))
    for ci in range(8):
        bi, cj = ci // NCL, ci % NCL
        b0, c0 = bi * BL, cj * CL
        ll = (lab[b0 : b0 + BL] - c0).reshape(RT, 128).T  # [128, RT]
        valid = (ll >= 0) & (ll < CL)
        labx = np.full((128, 2 * RT), -2, dtype=np.int16)
        labx[:, 0::2] = np.where(valid, ll, -1).astype(np.int16)
        maps.append(
            {
                "x": xbf[b0 : b0 + BL],
                "w": wbf[c0 : c0 + CL],
                "labx": np.ascontiguousarray(labx),
                "mvec": np.full((128, 1), mval, dtype=np.float32),
            }
        )
    return maps


def kernel(input, label, weight, s, m):
    from concourse.bass_utils import run_bass_kernel_spmd

    if "nc" not in _CACHE:
        _CACHE["nc"] = _build()
    nc = _CACHE["nc"]

    x = np.ascontiguousarray(np.asarray(input, dtype=np.float32))
    w = np.ascontiguousarray(np.asarray(weight, dtype=np.float32))
    lab = np.asarray(label)
    mval = float(np.asarray(m))

    res = run_bass_kernel_spmd(nc, _in_maps(x, w, lab, mval), core_ids=list(range(8)))

    logits = np.empty((B, C), dtype=np.float32)
    cosine = np.empty((B, C), dtype=np.float32)
    for ci in range(8):
        bi, cj = ci // NCL, ci % NCL
        b0, c0 = bi * BL, cj * CL
        logits[b0 : b0 + BL, c0 : c0 + CL] = np.asarray(
            res.results[ci]["logits"], dtype=np.float32
        )
        cosine[b0 : b0 + BL, c0 : c0 + CL] = np.asarray(
            res.results[ci]["cosine"], dtype=np.float32
        )
    return logits, cosine
